# revision 8
# baseline (speedup 1.0000x reference)
"""Trainium2 Bass kernel: Luong-style attention with predictive alignment.

Math (see reference):
    h_t    = x[:, -1, :]                                   [B, H]
    t      = tanh(h_t @ W_p);  aligned = S*sigmoid(t @ v_p)
    scores[b,s] = sum_h x[b,s,h] * u[b,h],  u[b] = W_a @ h_t[b]
        (algebraic rewrite of (x @ W_a) . h_t -- avoids the B*S*H*H einsum)
    attn   = softmax(scores) * exp(-(pos-aligned)^2 / sigma2)
    ctx[b] = sum_s attn[b,s] * x[b,s,:]
    out    = tanh(concat(ctx, h_t) @ W_v)

Sharding: data-parallel over batch. 8 cores x 4 batches each; weights
replicated per core.

Schedule (single priority-ordered DMA ring for all bulk traffic):
  W_aT | x b0 c0-3 | W_p | x b0 c4-7 | x b1 | W_v[H:] | x b2 | x b3 | W_v[:H]
W_v rows [H:2H] reuse W_aT's SBUF (after the last u-broadcast reads it) and
W_v rows [0:H] reuse W_p's; the W_v[:H] half streams after the last x tile so
x never waits, and its matmuls are the last accumulation group anyway.

Per-batch two-phase softmax: phase A covers chunks 0-6 (cols 0-13) with bias
m1 = max(scores[cols 0:14]); its exp/context matmuls run while chunk 7 is
still streaming.  Phase B (chunk 7, cols 14-15) reuses the same m1 bias and
accumulates into the same PSUM context, so after the last x tile lands only
2 score STTs + 2 small exps + 4 context matmuls + transpose/merge remain.
Exact softmax: e^{s-g2-m1}/sum(e^{s-m1}) == softmax(s)*gauss for any bias m1
(worst |s - m1| on this distribution is ~28, far below fp32 exp overflow).

The 1/Z normalization is folded into the single DVE write that scatters the
transposed context into combT's strided per-batch columns.
"""

import math
from contextlib import ExitStack

import numpy as np

import concourse.bass as bass
import concourse.bass_isa as bass_isa
import concourse.mybir as mybir
import concourse.tile as tile
from concourse import bacc
from concourse.bass_utils import run_bass_kernel_spmd

B, S, H, SIZE = 32, 2048, 1024, 1024
NCORES = 8
BPC = B // NCORES          # batches per core
NCH = 8                    # x chunks per batch
SCH = S // NCH             # 256 sequence positions per chunk
A = 2                      # sub-slices (128 s-positions each) per chunk
COLS = NCH * A             # 16 score columns per batch
NCOLS_A = 14               # phase-A columns (chunks 0-6)
F32 = mybir.dt.float32
F32R = mybir.dt.float32r
SIGMA_SQ = 2.0 * (S / 2.0 / 2.0) ** 2    # D = S//2; 2*(D/2)^2 = 524288
INV_SG = 1.0 / math.sqrt(SIGMA_SQ)

_CACHE = {}
TRACE = False


def _build():
    AF = mybir.ActivationFunctionType
    OP = mybir.AluOpType
    nc = bacc.Bacc()

    x_s = nc.dram_tensor("x_s", [BPC, S, H], F32, kind="ExternalInput")
    w_p = nc.dram_tensor("w_p", [H, H], F32, kind="ExternalInput")
    w_at = nc.dram_tensor("w_at", [H, H], F32, kind="ExternalInput")
    w_v = nc.dram_tensor("w_v", [2 * H, SIZE], F32, kind="ExternalInput")
    htk = nc.dram_tensor("htk", [128, 8 * BPC], F32, kind="ExternalInput")
    vrep = nc.dram_tensor("vrep", [BPC, H], F32, kind="ExternalInput")
    posd = nc.dram_tensor("pos", [128, COLS], F32, kind="ExternalInput")
    idd = nc.dram_tensor("ident", [128, 128], F32, kind="ExternalInput")
    outd = nc.dram_tensor("out", [BPC, SIZE], F32, kind="ExternalOutput")

    with tile.TileContext(nc) as tc, ExitStack() as ctx:
        const = ctx.enter_context(tc.tile_pool(name="const", bufs=1))
        wts = ctx.enter_context(tc.tile_pool(name="wts", bufs=1))
        xs = ctx.enter_context(tc.tile_pool(name="xs", bufs=12))
        ubcp = ctx.enter_context(tc.tile_pool(name="ubcp", bufs=3))
        ctxp = ctx.enter_context(tc.tile_pool(name="ctxp", bufs=2))
        prodp = ctx.enter_context(tc.tile_pool(name="prodp", bufs=2))
        small = ctx.enter_context(tc.tile_pool(name="small", bufs=2))
        psUbc = ctx.enter_context(
            tc.tile_pool(name="psUbc", bufs=1, space=bass.MemorySpace.PSUM)
        )
        psCtx = ctx.enter_context(
            tc.tile_pool(name="psCtx", bufs=1, space=bass.MemorySpace.PSUM)
        )
        psT = ctx.enter_context(
            tc.tile_pool(name="psT", bufs=1, space=bass.MemorySpace.PSUM)
        )
        psO = ctx.enter_context(
            tc.tile_pool(name="psO", bufs=1, space=bass.MemorySpace.PSUM)
        )
        dpool = ctx.enter_context(
            tc.tile_pool(name="dram", bufs=1, space=bass.MemorySpace.DRAM)
        )

        # ---- small inputs ride the gpsimd ring; bulk traffic owns sync ----
        combT = const.tile([128, 8 * BPC * 2], F32R)  # combined^T: [p, 4k+b]
        v_sb = const.tile([BPC, H], F32)
        pos_sb = const.tile([128, COLS], F32)
        id_sb = const.tile([128, 128], F32)
        tta = const.tile([BPC, H], F32)
        alb = const.tile([BPC, 1], F32)
        out_sb = const.tile([BPC, SIZE], F32)

        nc.gpsimd.dma_start(out=combT[:, 32:64], in_=htk[:, :].bitcast(F32R))
        nc.gpsimd.dma_start(out=v_sb, in_=vrep[:, :])
        nc.gpsimd.dma_start(out=pos_sb, in_=posd[:, :])
        nc.gpsimd.dma_start(out=id_sb, in_=idd[:, :])

        # ---- sync ring, strict priority order: W_aT first (u gates scores)
        wa_sb = wts.tile([128, 8, H], F32R, tag="w1")
        nc.sync.dma_start(
            out=wa_sb[:, 0:4, :],
            in_=w_at[0 : H // 2, :].rearrange("(k p) j -> p k j", p=128).bitcast(F32R),
        )
        nc.sync.dma_start(
            out=wa_sb[:, 4:8, :],
            in_=w_at[H // 2 :, :].rearrange("(k p) j -> p k j", p=128).bitcast(F32R),
        )

        # u[b] broadcast across partitions, computed directly on PE: lhsT is
        # the h_t column replicated along its free dim (step-0 AP), so
        # out[p, h] = sum_k h_t[b,k] W_aT[k,h] = u[b,h] for every partition p.
        ubc_tiles = [None] * BPC

        def emit_ubc(b):
            ub_ps = psUbc.tile([128, H], F32, tag="ub", name=f"ubps_{b}")
            for k in range(8):
                c0 = combT[:, 32 + 4 * k + b : 32 + 4 * k + b + 1]
                lhs = bass.AP(
                    tensor=c0.tensor, offset=c0.offset, ap=[c0.ap[0], [0, 128]]
                )
                for h2 in range(2):
                    nc.tensor.matmul(
                        ub_ps[:, 512 * h2 : 512 * (h2 + 1)],
                        lhs,
                        wa_sb[:, k, 512 * h2 : 512 * (h2 + 1)],
                        start=(k == 0),
                        stop=(k == 7),
                    )
            ubc = ubcp.tile([128, H], F32, tag="ubc", name=f"ubc_{b}")
            nc.scalar.copy(ubc, ub_ps)
            ubc_tiles[b] = ubc

        emit_ubc(0)
        emit_ubc(1)
        emit_ubc(2)
        emit_ubc(3)

        # ---- x DMAs (sync ring) ----
        all_x = [[None] * NCH for _ in range(BPC)]

        def emit_x_dmas(b, cs):
            for c in cs:
                xt = xs.tile([128, A, H], F32R, tag="xt", name=f"xt_{b}_{c}")
                nc.sync.dma_start(
                    out=xt,
                    in_=x_s[b, c * SCH : (c + 1) * SCH, :]
                    .rearrange("(p a) h -> p a h", p=128)
                    .bitcast(F32R),
                )
                all_x[b][c] = xt

        emit_x_dmas(0, range(4))

        # ---- W_p + t/aligned (f32r matmul; lhsT = combT h_t cols) ----
        wp_sb = wts.tile([128, 8, H], F32R, tag="w0")
        nc.sync.dma_start(
            out=wp_sb, in_=w_p[:, :].rearrange("(k p) j -> p k j", p=128).bitcast(F32R)
        )
        emit_x_dmas(0, range(4, NCH))
        ab_d = dpool.tile([BPC, 1], F32)

        def emit_aligned_section():
            # t = tanh(h_t @ W_p); aligned = S*sigmoid(t @ v_p)
            ps_t = psO.tile([BPC, H], F32, tag="po")
            for k in range(8):
                lhs = combT[:, 32 + 4 * k : 32 + 4 * k + 4]
                for h2 in range(2):
                    nc.tensor.matmul(
                        ps_t[:, 512 * h2 : 512 * (h2 + 1)],
                        lhs,
                        wp_sb[:, k, 512 * h2 : 512 * (h2 + 1)],
                        start=(k == 0),
                        stop=(k == 7),
                    )
            nc.scalar.activation(out=tta, in_=ps_t, func=AF.Tanh)

            prod0 = prodp.tile([BPC, H], F32, tag="p0")
            al_r = small.tile([BPC, 1], F32, tag="alr")
            nc.vector.scalar_tensor_tensor(
                out=prod0,
                in0=tta,
                scalar=1.0,
                in1=v_sb,
                op0=OP.mult,
                op1=OP.mult,
                accum_out=al_r,
            )
            nc.scalar.activation(out=alb, in_=al_r, func=AF.Sigmoid)
            nc.scalar.mul(alb, alb, -float(S) * INV_SG)  # alb = -aligned/sg
            nc.scalar.dma_start(out=ab_d[:, :], in_=alb)

        # ---- per-batch: scores + chunk-granular softmax/context ----
        # The bias m1 is fixed after chunk 3 (cols 0-7); every later chunk's
        # exp + 2 context matmuls run as soon as its score STT lands, so for
        # the last-streamed batch only chunk 7's sliver remains post-stream.
        # Exact softmax: e^{s-g2-m1}/sum(e^{s-m1}) == softmax(s)*gauss for
        # any bias m1 (worst |s-m1| here is far below fp32 exp overflow).
        NCH0 = 4          # chunks covered by the m1 bias phase
        C0 = NCH0 * A     # cols 0..7

        def batch_section(b, after_scores=None, mid_hook=None):
            ubc = ubc_tiles[b]
            sc_b = small.tile([128, COLS], F32, tag="scb", name=f"scb_{b}")

            def emit_stt(col):
                c, a = col // A, col % A
                prod = prodp.tile([128, H], F32, tag="p0", name=f"pr_{b}_{col}")
                nc.vector.scalar_tensor_tensor(
                    out=prod,
                    in0=all_x[b][c][:, a, :].bitcast(F32),
                    scalar=1.0,
                    in1=ubc,
                    op0=OP.mult,
                    op1=OP.mult,
                    accum_out=sc_b[:, col : col + 1],
                )

            for col in range(C0):
                emit_stt(col)
            if after_scores is not None:
                after_scores()

            # fix bias m1 = max over cols 0..7
            mx_p = small.tile([128, 1], F32, tag="mxp", name=f"mxp_{b}")
            nc.vector.reduce_max(
                out=mx_p, in_=sc_b[:, 0:C0], axis=mybir.AxisListType.X
            )
            mcast = small.tile([128, 1], F32, tag="mcast", name=f"mcast_{b}")
            nc.gpsimd.partition_all_reduce(
                mcast, mx_p, channels=128, reduce_op=bass_isa.ReduceOp.max
            )
            negm = small.tile([128, 1], F32, tag="negm", name=f"negm_{b}")
            nc.scalar.mul(negm, mcast, -1.0)

            ew0 = small.tile([128, C0], F32, tag="ew", name=f"ew0_{b}")
            zp0 = small.tile([128, 1], F32, tag="zp0", name=f"zp0_{b}")
            nc.scalar.activation(
                out=ew0,
                in_=sc_b[:, 0:C0],
                func=AF.Exp,
                bias=negm,
                scale=1.0,
                accum_out=zp0,
            )

            ab_b = small.tile([128, 1], F32, tag="abb", name=f"abb_{b}")
            nc.scalar.dma_start(
                out=ab_b, in_=ab_d[b : b + 1, :].to_broadcast((128, 1))
            )
            g2 = small.tile([128, COLS], F32, tag="g2", name=f"g2_{b}")
            nc.scalar.activation(
                out=g2, in_=pos_sb, func=AF.Square, bias=ab_b, scale=INV_SG
            )
            eb0 = small.tile([128, C0], F32, tag="eb0", name=f"eb0_{b}")
            nc.vector.tensor_sub(out=eb0, in0=sc_b[:, 0:C0], in1=g2[:, 0:C0])
            at0 = small.tile([128, C0], F32R, tag="at0", name=f"at0_{b}")
            nc.scalar.activation(out=at0, in_=eb0, func=AF.Exp, bias=negm, scale=1.0)

            ps_c = psCtx.tile([1, H], F32, tag="pc", name=f"pc_{b}")
            for col in range(C0):
                c, a = col // A, col % A
                for h2 in range(2):
                    nc.tensor.matmul(
                        ps_c[:, 512 * h2 : 512 * (h2 + 1)],
                        at0[:, col : col + 1],
                        all_x[b][c][:, a, 512 * h2 : 512 * (h2 + 1)],
                        start=(col == 0),
                        stop=False,
                    )
            if mid_hook is not None:
                # PE work that's ready during this batch's softmax window
                mid_hook()

            # chunks 4..7: exp + context matmuls fire per chunk as it lands
            zrun = zp0
            for ch in range(NCH0, NCH):
                col0 = ch * A
                for col in range(col0, col0 + A):
                    emit_stt(col)
                ewc = small.tile([128, A], F32, tag=f"ew{ch}", name=f"ew{ch}_{b}")
                zpc = small.tile([128, 1], F32, tag=f"zp{ch}", name=f"zp{ch}_{b}")
                nc.scalar.activation(
                    out=ewc,
                    in_=sc_b[:, col0 : col0 + A],
                    func=AF.Exp,
                    bias=negm,
                    scale=1.0,
                    accum_out=zpc,
                )
                ebc = small.tile([128, A], F32, tag=f"eb{ch}", name=f"eb{ch}_{b}")
                nc.vector.tensor_sub(
                    out=ebc, in0=sc_b[:, col0 : col0 + A], in1=g2[:, col0 : col0 + A]
                )
                atc = small.tile([128, A], F32R, tag=f"at{ch}", name=f"at{ch}_{b}")
                nc.scalar.activation(
                    out=atc, in_=ebc, func=AF.Exp, bias=negm, scale=1.0
                )
                for col in range(col0, col0 + A):
                    c, a = col // A, col % A
                    for h2 in range(2):
                        nc.tensor.matmul(
                            ps_c[:, 512 * h2 : 512 * (h2 + 1)],
                            atc[:, col - col0 : col - col0 + 1],
                            all_x[b][c][:, a, 512 * h2 : 512 * (h2 + 1)],
                            start=False,
                            stop=(col == COLS - 1 and h2 == 1),
                        )
                zn = small.tile([128, 1], F32, tag=f"zr{ch}", name=f"zr{ch}_{b}")
                nc.vector.tensor_add(out=zn, in0=zrun, in1=zpc)
                zrun = zn

            zsum = small.tile([128, 1], F32, tag="zsum", name=f"zsum_{b}")
            nc.gpsimd.partition_all_reduce(
                zsum, zrun, channels=128, reduce_op=bass_isa.ReduceOp.add
            )
            zinv = small.tile([128, 1], F32, tag="zinv", name=f"zinv_{b}")
            nc.vector.reciprocal(zinv, zsum)

            # ctx out of PSUM (split across scalar+vector), transpose
            # 128-blocks, then one strided DVE write folds 1/Z and lands
            # all 8 combT columns for this batch
            ctx_t = ctxp.tile([1, H], F32, tag="ctx", name=f"ctx_{b}")
            nc.scalar.copy(ctx_t[0:1, 0 : H // 2], ps_c[0:1, 0 : H // 2])
            nc.vector.tensor_copy(out=ctx_t[0:1, H // 2 : H], in_=ps_c[0:1, H // 2 : H])
            ps_ct = psT.tile([128, 8], F32, tag="pt", name=f"pct_{b}")
            for k in range(8):
                nc.tensor.transpose(
                    ps_ct[:, k : k + 1],
                    ctx_t[0:1, 128 * k : 128 * (k + 1)],
                    id_sb[0:1, 0:1],
                )
            cT = combT[:, b : b + 1]
            comb_cols = bass.AP(
                tensor=cT.tensor, offset=cT.offset, ap=[cT.ap[0], [4, 8]]
            )
            nc.vector.tensor_scalar_mul(comb_cols, ps_ct, zinv)

        # final-output accumulator [BPC, SIZE]; h_t-half matmuls run early
        ps_o = psO.tile([BPC, SIZE], F32, tag="po")

        def emit_final_hhalf():
            for k in range(8, 16):
                lhs = combT[:, 4 * k : 4 * k + 4]
                for h2 in range(2):
                    nc.tensor.matmul(
                        ps_o[:, 512 * h2 : 512 * (h2 + 1)],
                        lhs,
                        wv1_sb[:, k % 8, 512 * h2 : 512 * (h2 + 1)],
                        start=(k == 8),
                        stop=False,
                    )

        batch_section(0, after_scores=emit_aligned_section)
        emit_x_dmas(1, range(NCH))

        # W_v rows [H:2H] reuse W_aT's SBUF slot; DMA waits on ubc_3's reads
        # (all ubc broadcasts are emitted upfront, so that's ~25us in)
        wv1_sb = wts.tile([128, 8, SIZE], F32R, tag="w1")
        nc.sync.dma_start(
            out=wv1_sb,
            in_=w_v[H : 2 * H, :].rearrange("(k p) o -> p k o", p=128).bitcast(F32R),
        )
        batch_section(1)
        emit_x_dmas(2, range(NCH))
        batch_section(2, mid_hook=emit_final_hhalf)
        emit_x_dmas(3, range(NCH))

        # W_v rows [0:H] reuse W_p's slot; queued after all x so the x
        # stream is never stalled behind it (its matmuls are last anyway)
        wv0_sb = wts.tile([128, 8, SIZE], F32R, tag="w0")
        nc.sync.dma_start(
            out=wv0_sb[:, 0:4, :],
            in_=w_v[0 : H // 2, :].rearrange("(k p) o -> p k o", p=128).bitcast(F32R),
        )
        nc.sync.dma_start(
            out=wv0_sb[:, 4:8, :],
            in_=w_v[H // 2 : H, :].rearrange("(k p) o -> p k o", p=128).bitcast(F32R),
        )
        batch_section(3)

        for k in range(8):
            lhs = combT[:, 4 * k : 4 * k + 4]
            for h2 in range(2):
                nc.tensor.matmul(
                    ps_o[:, 512 * h2 : 512 * (h2 + 1)],
                    lhs,
                    wv0_sb[:, k, 512 * h2 : 512 * (h2 + 1)],
                    start=False,
                    stop=(k == 7 and h2 == 1),
                )
        # split tanh+store so the first half's DMA overlaps the second tanh
        nc.scalar.activation(
            out=out_sb[:, 0 : SIZE // 2], in_=ps_o[:, 0 : SIZE // 2], func=AF.Tanh
        )
        nc.sync.dma_start(
            out=outd[:, 0 : SIZE // 2], in_=out_sb[:, 0 : SIZE // 2]
        )
        nc.scalar.activation(
            out=out_sb[:, SIZE // 2 :], in_=ps_o[:, SIZE // 2 :], func=AF.Tanh
        )
        nc.sync.dma_start(out=outd[:, SIZE // 2 :], in_=out_sb[:, SIZE // 2 :])

    nc.compile()
    return nc


def _host_prep(x, W_p, v_p, W_a, W_v):
    x = np.ascontiguousarray(np.asarray(x, dtype=np.float32))
    W_p = np.ascontiguousarray(np.asarray(W_p, dtype=np.float32))
    v_p = np.asarray(v_p, dtype=np.float32).reshape(-1)
    W_aT = np.ascontiguousarray(np.asarray(W_a, dtype=np.float32).T)
    W_v = np.ascontiguousarray(np.asarray(W_v, dtype=np.float32))

    h_all = np.ascontiguousarray(x[:, -1, :])  # [B, H]
    vrep = np.ascontiguousarray(np.broadcast_to(v_p.reshape(1, H), (BPC, H)))
    cols = np.arange(COLS)
    p = np.arange(128)
    pos = ((cols[None, :] // A) * SCH + p[:, None] * A + (cols[None, :] % A)).astype(
        np.float32
    )
    pos = np.ascontiguousarray(pos)
    ident = np.eye(128, dtype=np.float32)

    in_maps = []
    for c in range(NCORES):
        hT = h_all[BPC * c : BPC * (c + 1)].T  # [H, BPC]
        htk_a = np.ascontiguousarray(
            hT.reshape(8, 128, BPC).transpose(1, 0, 2).reshape(128, 8 * BPC)
        )
        in_maps.append(
            dict(
                x_s=np.ascontiguousarray(x[BPC * c : BPC * (c + 1)]),
                w_p=W_p,
                w_at=W_aT,
                w_v=W_v,
                htk=htk_a,
                vrep=vrep,
                pos=pos,
                ident=ident,
            )
        )
    return in_maps


def kernel(x, W_p, v_p, W_a, W_v):
    if "nc" not in _CACHE:
        _CACHE["nc"] = _build()
    nc = _CACHE["nc"]
    in_maps = _host_prep(x, W_p, v_p, W_a, W_v)
    res = run_bass_kernel_spmd(nc, in_maps, core_ids=list(range(NCORES)), trace=TRACE)
    _CACHE["last_results"] = res
    return np.concatenate([r["out"] for r in res.results], axis=0)
